# revision 2
# baseline (speedup 1.0000x reference)
"""Trainium2 Bass kernel for 2-layer GRU (Keras reset_after) + 3 Dense layers, v2.

Model (per reference):
  h1 = GRU(x; k1, r1, b1)            # [B,T,64] -> [B,T,256], full sequence
  h2 = GRU(h1; k2, r2, b2)[:, -1]    # last state, [B,128]
  y  = ((h2 @ w3 + b3) @ w4 + b4) @ w5 + b5   # [B,24]

Pure data parallel over 8 NeuronCores (batch 256 -> 32 per core), transposed
layout (units on partitions, batch on free dim).

v2 restructuring vs v1 (both layers advance together, GRU2 one step behind):
  - Single fused elementwise chain per step. The per-step critical path is
    6 ops: sigmoid(r) -> mul(r*rh) -> add(+xh) -> tanh -> mul(w*hh) -> add.
  - GRU2's gate math rides along in the same wide instructions: PSUM gate
    tiles are laid out [g1 | g2] contiguous so one ACT/DVE op covers both.
  - Blend uses h' = z*h + w*hh with w = 1-z from negated z-weights and
    z = 1-w via a DVE tensor_scalar (no second sigmoid on the chain).
    a = z*h_prev runs in tanh's shadow; only mul+add remain after tanh.
  - GRU2-only leftovers (candidate add, blend) run on the idle Pool/GpSimd
    engine so they never block GRU1's DVE chain.
  - PSUM gate regions close early: per-step matmuls are ordered so the
    r-gate region is finished first (sigmoid starts while z/h matmuls run).
"""

import numpy as np

import concourse.bass as bass
import concourse.mybir as mybir
import concourse.tile as tile
from concourse import bacc
from concourse.bass_utils import run_bass_kernel_spmd

F16 = mybir.dt.float16
F32 = mybir.dt.float32
AF = mybir.ActivationFunctionType
OP = mybir.AluOpType

B, T_FULL, F = 256, 512, 64
U1, U2, OUT = 256, 128, 24
NCORES = 8
BL = B // NCORES  # 32 local batch


def _prep(inputs, T):
    """Host-side preprocessing -> (list of per-core input dicts, flags)."""
    x = np.asarray(inputs["x"], np.float32)[:, :T, :]
    k1 = np.asarray(inputs["k1"], np.float32)
    r1 = np.asarray(inputs["r1"], np.float32)
    b1 = np.asarray(inputs["b1"], np.float32)
    k2 = np.asarray(inputs["k2"], np.float32)
    r2 = np.asarray(inputs["r2"], np.float32)
    b2 = np.asarray(inputs["b2"], np.float32)
    w3 = np.asarray(inputs["w3"], np.float32)
    b3 = np.asarray(inputs["b3"], np.float32)
    w4 = np.asarray(inputs["w4"], np.float32)
    b4 = np.asarray(inputs["b4"], np.float32)
    w5 = np.asarray(inputs["w5"], np.float32)
    b5 = np.asarray(inputs["b5"], np.float32)

    # gate column order in k/r: [z | r | h], each U wide
    # x-side z/r weights with bias row; z negated so sigmoid gives w = 1-z.
    bzr1 = b1[0] + b1[1]
    wk1_r = np.concatenate([k1[:, U1 : 2 * U1], bzr1[None, U1 : 2 * U1]], 0)  # [65,256]
    wk1_z = -np.concatenate([k1[:, :U1], bzr1[None, :U1]], 0)  # [65,256]
    wk1zr = np.concatenate([wk1_r, wk1_z], 1)  # [65, 512]: tiles r0,r1,zn0,zn1

    # x-side h candidate weights + input bias row (bulk precompute)
    wk1h = np.concatenate([k1[:, 2 * U1 :], b1[0][None, 2 * U1 :]], 0)  # [65,256]

    # recurrent r1 tiles [128,128]: order r(m,k) 0-3, zneg(m,k) 4-7, h(m,k) 8-11
    def r1_tile(goff, m, k, neg=False):
        t_ = r1[k * 128 : (k + 1) * 128, goff + m * 128 : goff + (m + 1) * 128]
        return -t_ if neg else t_

    tiles = []
    for goff, neg in ((U1, False), (0, True), (2 * U1, False)):
        for m in range(2):
            for k in range(2):
                tiles.append(r1_tile(goff, m, k, neg))
    wr1 = np.concatenate(tiles, 1)  # [128, 1536]

    # k2 tiles [128,128]: order r(k0,k1), zneg(k0,k1), h(k0,k1)
    tiles = []
    for goff, neg in ((U2, False), (0, True), (2 * U2, False)):
        for k in range(2):
            t_ = k2[k * 128 : (k + 1) * 128, goff : goff + U2]
            tiles.append(-t_ if neg else t_)
    wk2 = np.concatenate(tiles, 1)  # [128, 768]

    # r2: [r2_r | -r2_z | r2_h]
    wr2 = np.concatenate([r2[:, U2 : 2 * U2], -r2[:, :U2], r2[:, 2 * U2 :]], 1)

    bzr2 = b2[0] + b2[1]
    wb2zr = np.concatenate([bzr2[U2 : 2 * U2], -bzr2[:U2]])[None, :]  # [1,256] r|zneg
    wb2xh = b2[0][None, 2 * U2 :]  # [1,128]
    vb2rh = b2[1][2 * U2 :, None]  # [128,1]
    vb1h = np.stack([b1[1, 2 * U1 : 2 * U1 + 128], b1[1, 2 * U1 + 128 :]], 1)  # [128,2]

    vbd = np.zeros((128, 3), np.float32)
    vbd[:64, 0] = b3
    vbd[:32, 1] = b4
    vbd[:OUT, 2] = b5

    flags = {
        "HAS_B1H": bool(np.any(b1[1, 2 * U1 :] != 0)),
        "HAS_B2ZR": bool(np.any(bzr2[: 2 * U2] != 0)),
        "HAS_B2XH": bool(np.any(b2[0, 2 * U2 :] != 0)),
        "HAS_B2RH": bool(np.any(b2[1, 2 * U2 :] != 0)),
    }

    shared = {
        "wk1zr": wk1zr.astype(np.float16),
        "wk1h": wk1h.astype(np.float16),
        "wr1": wr1.astype(np.float16),
        "wk2": wk2.astype(np.float16),
        "wr2": wr2.astype(np.float16),
        "wb2zr": wb2zr.astype(np.float16),
        "wb2xh": wb2xh.astype(np.float16),
        "vb2rh": vb2rh.astype(np.float32),
        "vb1h": vb1h.astype(np.float32),
        "vbd": vbd.astype(np.float32),
        "wd3": w3.astype(np.float16),
        "wd4": w4.astype(np.float16),
        "wd5": w5.astype(np.float16),
    }

    in_maps = []
    for c in range(NCORES):
        xs = x[c * BL : (c + 1) * BL]  # [BL, T, F]
        xt = np.ascontiguousarray(xs.transpose(2, 1, 0)).reshape(F, T * BL)
        xin = np.concatenate([xt, np.ones((1, T * BL), np.float32)], 0)
        m = dict(shared)
        m["xin"] = xin.astype(np.float16)
        in_maps.append(m)
    return in_maps, flags


def _build(T, flags, debug_h=False):
    """Emit the Bass program for T timesteps. Returns compiled nc."""
    HAS_B1H = flags["HAS_B1H"]
    HAS_B2ZR = flags["HAS_B2ZR"]
    HAS_B2XH = flags["HAS_B2XH"]
    HAS_B2RH = flags["HAS_B2RH"]
    nc = bacc.Bacc("TRN2", target_bir_lowering=False, debug=False, num_devices=NCORES)

    d_xin = nc.dram_tensor("xin", [F + 1, T * BL], F16, kind="ExternalInput").ap()
    d_wk1zr = nc.dram_tensor("wk1zr", [F + 1, 512], F16, kind="ExternalInput").ap()
    d_wk1h = nc.dram_tensor("wk1h", [F + 1, 256], F16, kind="ExternalInput").ap()
    d_wr1 = nc.dram_tensor("wr1", [128, 1536], F16, kind="ExternalInput").ap()
    d_wk2 = nc.dram_tensor("wk2", [128, 768], F16, kind="ExternalInput").ap()
    d_wr2 = nc.dram_tensor("wr2", [128, 384], F16, kind="ExternalInput").ap()
    d_wb2zr = nc.dram_tensor("wb2zr", [1, 256], F16, kind="ExternalInput").ap()
    d_wb2xh = nc.dram_tensor("wb2xh", [1, 128], F16, kind="ExternalInput").ap()
    d_vb2rh = nc.dram_tensor("vb2rh", [128, 1], F32, kind="ExternalInput").ap()
    d_vb1h = nc.dram_tensor("vb1h", [128, 2], F32, kind="ExternalInput").ap()
    d_vbd = nc.dram_tensor("vbd", [128, 3], F32, kind="ExternalInput").ap()
    d_wd3 = nc.dram_tensor("wd3", [128, 64], F16, kind="ExternalInput").ap()
    d_wd4 = nc.dram_tensor("wd4", [64, 32], F16, kind="ExternalInput").ap()
    d_wd5 = nc.dram_tensor("wd5", [32, OUT], F16, kind="ExternalInput").ap()
    d_y = nc.dram_tensor("y", [BL, OUT], F32, kind="ExternalOutput").ap()
    d_hdbg = (
        nc.dram_tensor("hdbg", [2, 128, 96], F16, kind="ExternalOutput").ap()
        if debug_h
        else None
    )
    dbg_tiles = {}
    d_dbg = {}
    if debug_h:
        for nm, w, dt_ in [("S_r", 96, F16), ("S_w", 96, F16), ("T1", 96, F16),
                           ("PRE", 64, F16), ("HH1", 64, F16), ("Z", 96, F16),
                           ("A", 96, F16), ("B1", 64, F16),
                           ("P", 192, F32), ("Q", 128, F32)]:
            d_dbg[nm] = nc.dram_tensor(
                f"dbg_{nm}", [128, w], dt_, kind="ExternalOutput"
            ).ap()

    with tile.TileContext(nc) as tc:
        with (
            tc.tile_pool(name="big", bufs=1) as big,
            tc.tile_pool(name="wts", bufs=1) as wts,
            tc.tile_pool(name="state", bufs=1) as state,
            tc.tile_pool(name="tmp", bufs=6) as tmp,
        ):
            sb_x = big.tile([F + 1, T * BL], F16, tag="sb_x", name="sb_x")
            sb_xg1h = big.tile([128, T, 64], F16, tag="sb_xg1h", name="sb_xg1h")

            def wtile(name, shape, dt, src):
                t_ = wts.tile(shape, dt, tag=name, name=name)
                nc.sync.dma_start(out=t_[:], in_=src[:])
                return t_

            sb_wk1zr = wtile("sb_wk1zr", [F + 1, 512], F16, d_wk1zr)
            sb_wk1h = wtile("sb_wk1h", [F + 1, 256], F16, d_wk1h)
            sb_wr1 = wtile("sb_wr1", [128, 1536], F16, d_wr1)
            sb_wk2 = wtile("sb_wk2", [128, 768], F16, d_wk2)
            sb_wr2 = wtile("sb_wr2", [128, 384], F16, d_wr2)
            sb_vbd = wtile("sb_vbd", [128, 3], F32, d_vbd)
            sb_wd3 = wtile("sb_wd3", [128, 64], F16, d_wd3)
            sb_wd4 = wtile("sb_wd4", [64, 32], F16, d_wd4)
            sb_wd5 = wtile("sb_wd5", [32, OUT], F16, d_wd5)
            sb_wb2zr = wtile("sb_wb2zr", [1, 256], F16, d_wb2zr) if HAS_B2ZR else None
            sb_wb2xh = wtile("sb_wb2xh", [1, 128], F16, d_wb2xh) if HAS_B2XH else None
            sb_vb2rh = wtile("sb_vb2rh", [128, 1], F32, d_vb2rh) if HAS_B2RH else None
            sb_vb1h = wtile("sb_vb1h", [128, 2], F32, d_vb1h) if HAS_B1H else None

            sb_ones = None
            if HAS_B2ZR or HAS_B2XH:
                sb_ones = wts.tile([1, BL], F16, tag="sb_ones", name="sb_ones")
                nc.vector.memset(sb_ones[:], 1.0)

            # h-state tiles: H[p] = [h1(t) | h2(t-1)] for wall step t = p (mod 2)
            H = [
                state.tile([128, 96], F16, tag=f"H{i}", name=f"H{i}") for i in range(2)
            ]
            nc.vector.memset(H[0][:], 0.0)
            nc.vector.memset(H[1][:], 0.0)

            # x load, split across a few DMAs
            nchunk = 4
            cw = (T * BL) // nchunk
            for i in range(nchunk):
                nc.sync.dma_start(
                    out=sb_x[:, i * cw : (i + 1) * cw],
                    in_=d_xin[:, i * cw : (i + 1) * cw],
                )

            # ---- bulk precompute xg1h = [x;1] @ [k1_h; b1_0h]  -> sb_xg1h ----
            with tc.tile_pool(name="bulkps", bufs=2, space="PSUM") as bulkps:
                CH = 16  # timesteps per matmul (N = CH*BL = 512)
                for ci in range((T + CH - 1) // CH):
                    t0 = ci * CH
                    ts_ = min(CH, T - t0)
                    n = ts_ * BL
                    for m in range(2):
                        pb = bulkps.tile([128, 512], F32, tag="pb", name="pb")
                        nc.tensor.matmul(
                            pb[:, :n],
                            sb_wk1h[:, m * 128 : (m + 1) * 128],
                            sb_x[:, t0 * BL : t0 * BL + n],
                            start=True,
                            stop=True,
                        )
                        dst = sb_xg1h[:, t0 : t0 + ts_, m * 32 : (m + 1) * 32]
                        src = pb.rearrange("p (t b) -> p t b", b=BL)[:, :ts_, :]
                        if m == 0:
                            nc.vector.tensor_copy(dst, src)
                        else:
                            nc.scalar.copy(dst, src)

            # ---- the scan ----
            # PSUM layout per parity p:
            #   P[p] [128,192] f32: [g1_r(0:64) | g2_r(64:96) | g1_zn(96:160) | g2_zn(160:192)]
            #   Q[p] [128,128] f32: [g1_ph(0:64) | g2_rh(64:96) | g2_xh(96:128)]
            with tc.tile_pool(name="ps", bufs=1, space="PSUM") as psp:
                Pr = [
                    psp.tile([128, 96], F32, tag=f"Pr{i}", name=f"Pr{i}")
                    for i in range(2)
                ]
                Pz = [
                    psp.tile([128, 96], F32, tag=f"Pz{i}", name=f"Pz{i}")
                    for i in range(2)
                ]
                Q = [
                    psp.tile([128, 128], F32, tag=f"Q{i}", name=f"Q{i}")
                    for i in range(2)
                ]

                def mm(out, lhsT, rhs, start, stop):
                    nc.tensor.matmul(out, lhsT, rhs, start=start, stop=stop)

                # PSUM accumulation rule (empirical): the first-touch-
                # overwrite tracking that start=True opens does not span PSUM
                # tiles/banks. Each PSUM tile therefore gets its own
                # start=True on its first write of every step, all writes to
                # one tile are contiguous in the PE queue, and a tile's
                # init->accumulate window never contains another start.

                def emit_P_r(t):
                    """Pr[t%2] writers: xg_r(START), g1_r, k2_r, [b2zr],
                    r2_r. Opens the Pr tile's accumulation group."""
                    Pp = Pr[t % 2]
                    g1 = t < T
                    has_k2 = t >= 1
                    has_r2 = t >= 2
                    h1p = H[(t - 1) % 2]
                    h2p = H[(t - 1) % 2][:, 64:96]
                    first = True
                    if g1:
                        rhs = sb_x[:, t * BL : (t + 1) * BL]
                        solo = t == 0
                        for m in range(2):
                            mm(Pp[:, m * 32 : (m + 1) * 32],
                               sb_wk1zr[:, m * 128 : (m + 1) * 128], rhs,
                               first, solo)
                            first = False
                        if t >= 1:
                            for m in range(2):
                                for k in range(2):
                                    mm(Pp[:, m * 32 : (m + 1) * 32],
                                       sb_wr1[:, (m * 2 + k) * 128 : (m * 2 + k + 1) * 128],
                                       h1p[:, k * 32 : (k + 1) * 32], False, k == 1)
                    if has_k2:
                        for k in range(2):
                            mm(Pp[:, 64:96], sb_wk2[:, k * 128 : (k + 1) * 128],
                               h1p[:, k * 32 : (k + 1) * 32], first,
                               k == 1 and not has_r2 and not HAS_B2ZR)
                            first = False
                        if HAS_B2ZR:
                            mm(Pp[:, 64:96], sb_wb2zr[:, 0:128], sb_ones[:],
                               False, not has_r2)
                        if has_r2:
                            if g2split[0] is not None:
                                # h2(t-2) = A_prev[64:96] + B2_prev: two matmuls
                                # so the late addh2 is never on this path
                                mm(Pp[:, 64:96], sb_wr2[:, 0:128],
                                   g2split[0][:, 64:96], False, False)
                                mm(Pp[:, 64:96], sb_wr2[:, 0:128],
                                   g2split[1][:], False, True)
                            else:
                                mm(Pp[:, 64:96], sb_wr2[:, 0:128], h2p, False, True)

                def emit_P_z(t):
                    """Pz[t%2] writers: xg_z(START), g1_zn, k2_zn, [b2zr],
                    r2_zn. Own tile -> own accumulation group."""
                    Pp = Pz[t % 2]
                    g1 = t < T
                    has_k2 = t >= 1
                    has_r2 = t >= 2
                    h1p = H[(t - 1) % 2]
                    h2p = H[(t - 1) % 2][:, 64:96]
                    first = True
                    if g1:
                        rhs = sb_x[:, t * BL : (t + 1) * BL]
                        solo = t == 0
                        for m in range(2):
                            mm(Pp[:, m * 32 : (m + 1) * 32],
                               sb_wk1zr[:, 256 + m * 128 : 384 + m * 128], rhs,
                               first, solo)
                            first = False
                        if t >= 1:
                            for m in range(2):
                                for k in range(2):
                                    mm(Pp[:, m * 32 : (m + 1) * 32],
                                       sb_wr1[:, (4 + m * 2 + k) * 128 : (5 + m * 2 + k) * 128],
                                       h1p[:, k * 32 : (k + 1) * 32], False, k == 1)
                    if has_k2:
                        for k in range(2):
                            mm(Pp[:, 64:96], sb_wk2[:, (2 + k) * 128 : (3 + k) * 128],
                               h1p[:, k * 32 : (k + 1) * 32], first,
                               k == 1 and not has_r2 and not HAS_B2ZR)
                            first = False
                        if HAS_B2ZR:
                            mm(Pp[:, 64:96], sb_wb2zr[:, 128:256], sb_ones[:],
                               False, not has_r2)
                        if has_r2:
                            if g2split[0] is not None:
                                mm(Pp[:, 64:96], sb_wr2[:, 128:256],
                                   g2split[0][:, 64:96], False, False)
                                mm(Pp[:, 64:96], sb_wr2[:, 128:256],
                                   g2split[1][:], False, True)
                            else:
                                mm(Pp[:, 64:96], sb_wr2[:, 128:256], h2p, False, True)

                def emit_Q_ph_rh(t):
                    """Q[t%2]: ph(START) + r2_h — the T1 inputs."""
                    Qp = Q[t % 2]
                    g1 = t < T
                    has_r2 = t >= 2
                    h1p = H[(t - 1) % 2]
                    h2p = H[(t - 1) % 2][:, 64:96]
                    first = True
                    if g1 and t >= 1:
                        for m in range(2):
                            for k in range(2):
                                mm(Qp[:, m * 32 : (m + 1) * 32],
                                   sb_wr1[:, (8 + m * 2 + k) * 128 : (9 + m * 2 + k) * 128],
                                   h1p[:, k * 32 : (k + 1) * 32], first, k == 1)
                                first = False
                    if has_r2:
                        if g2split[0] is not None:
                            mm(Qp[:, 64:96], sb_wr2[:, 256:384],
                               g2split[0][:, 64:96], first, False)
                            mm(Qp[:, 64:96], sb_wr2[:, 256:384],
                               g2split[1][:], False, True)
                        else:
                            mm(Qp[:, 64:96], sb_wr2[:, 256:384], h2p, first, True)
                        first = False
                    return first

                def emit_Q_xh(t, first):
                    """Q[t%2]: k2_h (+ bias) — the PRE2 input. Emitted after
                    Pz's start, so it opens its own group on the Q tile."""
                    Qp = Q[t % 2]
                    has_k2 = t >= 1
                    first = True
                    h1p = H[(t - 1) % 2]
                    if has_k2:
                        if HAS_B2XH:
                            mm(Qp[:, 96:128], sb_wb2xh[:, 0:128], sb_ones[:],
                               first, False)
                            first = False
                        for k in range(2):
                            mm(Qp[:, 96:128], sb_wk2[:, (4 + k) * 128 : (5 + k) * 128],
                               h1p[:, k * 32 : (k + 1) * 32], first, k == 1)
                            first = False

                def tl(name, w=96, dt=F16):
                    return tmp.tile([128, w], dt, tag=name, name=name)

                g2split = [None, None]  # prev step's (A, B2) for h2 split
                for t in range(T + 1):
                    g1 = t < T        # g1 step t active
                    g2 = t >= 1       # g2 step t-1 active
                    g2r = t >= 2      # g2 recurrent part active (s >= 1)
                    p = t % 2
                    Qp = Q[p]
                    Prp, Pzp = Pr[p], Pz[p]
                    hprev = H[(t - 1) % 2]

                    # ---- PE r-writers, then sigmoid(r) (tight watermark) ----
                    emit_P_r(t)
                    S_r = tl("S_r")
                    if g1 and t >= 1:
                        nc.scalar.activation(S_r[:, 0:96], Prp[:, 0:96], AF.Sigmoid)
                    elif g1:  # t == 0
                        nc.scalar.activation(S_r[:, 0:64], Prp[:, 0:64], AF.Sigmoid)
                    else:  # flush
                        nc.scalar.activation(S_r[:, 64:96], Prp[:, 64:96], AF.Sigmoid)

                    emit_P_z(t)
                    S_w = tl("S_w")
                    if g1 and t >= 1:
                        nc.scalar.activation(S_w[:, 0:96], Pzp[:, 0:96], AF.Sigmoid)
                    elif g1:
                        nc.scalar.activation(S_w[:, 0:64], Pzp[:, 0:64], AF.Sigmoid)
                    else:
                        nc.scalar.activation(S_w[:, 64:96], Pzp[:, 64:96], AF.Sigmoid)

                    qfirst = emit_Q_ph_rh(t)

                    # ---- DVE chain ----
                    T1 = tl("T1")
                    if t >= 1:
                        lo, hi = (0, 96) if (g1 and g2r) else ((0, 64) if g1 else (64, 96))
                        if HAS_B1H and g1:
                            for i in range(2):
                                nc.vector.scalar_tensor_tensor(
                                    T1[:, i * 32 : (i + 1) * 32],
                                    Qp[:, i * 32 : (i + 1) * 32],
                                    sb_vb1h[:, i : i + 1],
                                    S_r[:, i * 32 : (i + 1) * 32],
                                    OP.add, OP.mult)
                            if g2r:
                                if HAS_B2RH:
                                    nc.vector.scalar_tensor_tensor(
                                        T1[:, 64:96], Qp[:, 64:96], sb_vb2rh[:, 0:1],
                                        S_r[:, 64:96], OP.add, OP.mult)
                                else:
                                    nc.vector.tensor_mul(
                                        T1[:, 64:96], Qp[:, 64:96], S_r[:, 64:96])
                        elif HAS_B2RH and g2r:
                            if lo < 64:
                                nc.vector.tensor_mul(
                                    T1[:, lo:64], Qp[:, lo:64], S_r[:, lo:64])
                            nc.vector.scalar_tensor_tensor(
                                T1[:, 64:96], Qp[:, 64:96], sb_vb2rh[:, 0:1],
                                S_r[:, 64:96], OP.add, OP.mult)
                        else:
                            nc.vector.tensor_mul(
                                T1[:, lo:hi], Qp[:, lo:hi], S_r[:, lo:hi])

                    emit_Q_xh(t, qfirst)

                    PRE = tl("PRE", 64)
                    if g1 and t >= 1:
                        nc.vector.tensor_add(PRE[:], T1[:, 0:64], sb_xg1h[:, t, :])

                    # ---- DVE: g2 candidate pre-activation (PSUM src) ----
                    PRE2 = tl("PRE2", 32)
                    if g2r:
                        nc.vector.tensor_add(PRE2[:], T1[:, 64:96], Qp[:, 96:128])

                    # ---- DVE: z = 1 - w (in tanh's shadow) ----
                    Z = tl("Z")
                    if t >= 1:
                        lo, hi = (0, 96) if g1 else (64, 96)
                        nc.vector.tensor_scalar(
                            Z[:, lo:hi], S_w[:, lo:hi], -1.0, 1.0, OP.mult, OP.add)

                    # ---- Pool: a = z*h_prev (tanh shadow) ----
                    A = tl("A")
                    if t >= 1:
                        lo, hi = (0, 96) if g1 else (64, 96)
                        nc.vector.tensor_mul(
                            A[:, lo:hi], Z[:, lo:hi], hprev[:, lo:hi])

                    # ---- ACT: tanh ----
                    HH1 = tl("HH1", 64)
                    if g1:
                        src = PRE[:] if t >= 1 else sb_xg1h[:, 0, :]
                        nc.scalar.activation(HH1[:], src, AF.Tanh)
                    HH2 = tl("HH2", 32)
                    if g2:
                        src2 = PRE2[:] if g2r else Qp[:, 96:128]
                        nc.scalar.activation(HH2[:], src2, AF.Tanh)

                    # ---- DVE: g1 blend ----
                    if g1:
                        B1 = tl("B1", 64)
                        if t >= 1:
                            nc.vector.tensor_mul(B1[:], S_w[:, 0:64], HH1[:])
                            nc.vector.tensor_add(
                                H[p][:, 0:64], A[:, 0:64], B1[:])
                        else:
                            nc.vector.tensor_mul(
                                H[p][:, 0:64], S_w[:, 0:64], HH1[:])

                    # ---- Pool: g2 blend ----
                    if g2:
                        B2 = tl("B2", 32)
                        if g2r:
                            nc.vector.tensor_mul(B2[:], S_w[:, 64:96], HH2[:])
                            nc.vector.tensor_add(
                                H[p][:, 64:96], A[:, 64:96], B2[:])
                        else:
                            # s=0: h2(-1)=0 -> h2 = w*hh
                            nc.vector.tensor_mul(
                                H[p][:, 64:96], S_w[:, 64:96], HH2[:])

                    g2split = [A, B2] if g2r else [None, None]

                    if debug_h and t == 1:
                        dbgP = tmp.tile([128, 192], F32, tag="dbgP", name="dbgP")
                        dbgQ = tmp.tile([128, 128], F32, tag="dbgQ", name="dbgQ")
                        nc.scalar.copy(dbgP[:, 0:96], Prp[:])
                        nc.scalar.copy(dbgP[:, 96:192], Pzp[:])
                        nc.scalar.copy(dbgQ[:], Qp[:])
                        dbg_tiles.update(
                            S_r=S_r, S_w=S_w, T1=T1, PRE=PRE, HH1=HH1,
                            Z=Z, A=A, B1=B1, P=dbgP, Q=dbgQ,
                        )

                # ---- dense tail ----
                pd = Q[(T + 1) % 2]
                h2f = H[T % 2][:, 64:96]
                q3 = tmp.tile([64, 32], F16, tag="q3", name="q3")
                q4 = tmp.tile([32, 32], F16, tag="q4", name="q4")
                q5 = tmp.tile([32, 32], F32, tag="q5", name="q5")
                qt = tmp.tile([32, 32], F32, tag="qt", name="qt")
                nc.vector.memset(q5[:], 0.0)
                nc.tensor.matmul(pd[0:64, 0:32], sb_wd3[:], h2f[:], start=True, stop=True)
                nc.scalar.activation(
                    q3[:], pd[0:64, 0:32], AF.Identity, bias=sb_vbd[0:64, 0:1]
                )
                nc.tensor.matmul(pd[0:32, 32:64], sb_wd4[:], q3[:], start=True, stop=True)
                nc.scalar.activation(
                    q4[:], pd[0:32, 32:64], AF.Identity, bias=sb_vbd[0:32, 1:2]
                )
                nc.tensor.matmul(pd[0:OUT, 64:96], sb_wd5[:], q4[:], start=True, stop=True)
                nc.scalar.activation(
                    q5[0:OUT, :], pd[0:OUT, 64:96], AF.Identity, bias=sb_vbd[0:OUT, 2:3]
                )
                nc.vector.transpose(qt[:], q5[:])
                nc.sync.dma_start(out=d_y[:], in_=qt[0:BL, 0:OUT])
                if debug_h:
                    nc.sync.dma_start(out=d_hdbg[0], in_=H[0][:])
                    nc.sync.dma_start(out=d_hdbg[1], in_=H[1][:])
                    for nm, t_ in dbg_tiles.items():
                        nc.sync.dma_start(out=d_dbg[nm][:], in_=t_[:])

    nc.compile()
    return nc


def _run(inputs, T):
    in_maps, flags = _prep(inputs, T)
    nc = _build(T, flags)
    res = run_bass_kernel_spmd(nc, in_maps, core_ids=list(range(NCORES)))
    return np.concatenate([res.results[c]["y"] for c in range(NCORES)], 0).astype(
        np.float32
    )


def kernel(**inputs):
    return _run(inputs, T_FULL)
